# revision 22
# baseline (speedup 1.0000x reference)
"""Distributed Trainium2 Bass kernel for nn_AdjConv (gnn_message_passing).

Full (unsharded) inputs in, full output out. Internally shards the vertex
dim N=4096 across 8 NeuronCores (512 rows each) for the d/attention path
and the output, and shards the hyperedge dim E=1024 (128 edges/core) for
the s path.

Key algebraic optimization: the hadamard_power argument q/800 is tiny
(|q|<~20: s,d are LayerNormed, w is Xavier-scaled), so
  ta = exp(-q/800) = 1 - q/800 + O(3e-4)
to a relative accuracy far below the bf16 noise floor.  The linearized ta
is exactly rank R = H+2 = 66:
  ta[e,n] = U[e] . V[n]
  U = [ (2/800)*w .* s,  1 - b/800 - ssw/800,  1       ]   (E x 66)
  V = [ d,               1,                    -ddw/800 ]   (N x 66)
with ssw[e] = sum_k w_k s[e,k]^2, ddw[n] = sum_k w_k d[n,k]^2.  Then
  DV = V @ Usum,  DE = U @ Vsum,  K = U^T diag(1/DE) U   (66 x 66)
  out = 0.99 G + Vt (0.01 K) Vt^T     with Vt = DV^-1/2 * V
which turns the E=1024-contraction (N,E)@(E,N) matmul into a rank-66
contraction and eliminates the full-size exp(ta) evaluation entirely.
The only remaining exp is the softmax numerator exp(f f^T / 8) (the
softmax denominator and the /adj.sum(0) division cancel inside the two
LayerNorms; LN is invariant to positive row scaling).

Distribution: ONE collective.  The s path is e-sharded (feats is
replicated, adj is column-sharded), so s/U/Usum partials need no
communication; a single AllGather moves
  [ vT (66 x 512) | U_r (128 x 66) | Usum_r (66) | Vsum_r (66) ]  bf16
(~85KB/rank).  Everything before it overlaps the collective-init barrier;
after it each core assembles U/V globals from per-rank slice DMAs
(parallel DMA queues, no big single-queue reshuffle) and runs the rank-66
output matmul fused with the G add.

G rows are pre-scaled by 0.99 and cast bf16 on host; output is written
bf16 and upcast on host (G_new is ~1e-4 of output norm, so bf16 I/O costs
~1.7e-3 relative error against a 2e-2 budget).
"""
import numpy as np
import ml_dtypes

import concourse.bass as bass
import concourse.bacc as bacc
import concourse.mybir as mybir
from concourse import tile
from concourse.bass_utils import run_bass_kernel_spmd

BF = ml_dtypes.bfloat16
F32 = np.float32
DT_BF = mybir.dt.bfloat16
DT_F32 = mybir.dt.float32
MULT = mybir.AluOpType.mult
ADD = mybir.AluOpType.add
SUB = mybir.AluOpType.subtract
EXP = mybir.ActivationFunctionType.Exp
SQRT = mybir.ActivationFunctionType.Sqrt

N, E, D, H = 4096, 1024, 256, 64
NC = 8           # cores
NL = N // NC     # 512 local rows
EL = E // NC     # 128 local edges
P = 128
R = H + 2        # 66: low-rank width of linearized ta
DKT = D // P     # 2 d-chunks
KT = N // P      # 32 n' tiles
NKT = NL // P    # 4 local row tiles
NB = 512         # psum column block
NBT = N // NB    # 8
NQ = N // P      # 32 row-groups in replicated feats layout

LN_EPS = 1e-5
# u-gather (early): [U_r (128 x 66) | Usum_r (66)]
OU = 0
OUS = OU + P * R            # 8448
AG1 = OUS + R               # 8514
# v-gather (late): [vT|Vsum as (66 x 513) | invdvl (512)]
NL1 = NL + 1                # 513: vT cols + Vsum column
OV = 0
OVS = OV + R * NL1          # 33858
AG2 = OVS + NL              # 34370


def build_kernel(debug_taps=False):
    nc = bacc.Bacc("TRN2", target_bir_lowering=False, debug=False,
                   num_devices=NC)

    # ---- per-core external I/O -------------------------------------------
    # adjc: adj column shard (N x EL), row-grouped so partition p holds
    # rows p*32..p*32+32 (contiguous per partition).  featsn: full feats
    # (N x D) in the same row grouping.
    adjc_e = nc.dram_tensor("adjc", [P, NQ * EL], DT_BF, kind="ExternalInput")
    featsn_e = nc.dram_tensor("featsn", [P, NQ * D], DT_BF,
                              kind="ExternalInput")
    g_e = nc.dram_tensor("g", [NL, N], DT_BF, kind="ExternalInput")
    featsTf_e = nc.dram_tensor("featsTf", [D, N], DT_BF, kind="ExternalInput")
    featsTl_e = nc.dram_tensor("featsTl", [D, NL], DT_BF, kind="ExternalInput")
    wvT_e = nc.dram_tensor("wvT", [D, H], DT_BF, kind="ExternalInput")
    linT_e = nc.dram_tensor("linT", [D, H], DT_BF, kind="ExternalInput")
    wcol_e = nc.dram_tensor("wcol", [H, 1], DT_BF, kind="ExternalInput")
    ln2_e = nc.dram_tensor("ln2", [H, 2], DT_F32, kind="ExternalInput")
    rows4_e = nc.dram_tensor("rows4", [1, 4 * H], DT_BF, kind="ExternalInput")
    c1col_e = nc.dram_tensor("c1col", [P, 1], DT_F32, kind="ExternalInput")
    n800_e = nc.dram_tensor("n800col", [P, 1], DT_F32, kind="ExternalInput")
    ident_e = nc.dram_tensor("ident", [P, P], DT_BF, kind="ExternalInput")
    out_e = nc.dram_tensor("out", [NL, N], DT_BF, kind="ExternalOutput")

    # ---- internal DRAM (collective bounce buffers) -----------------------
    ag1_in = nc.dram_tensor("ag1_in", [AG1], DT_BF)
    ag1_out = nc.dram_tensor("ag1_out", [NC, AG1], DT_BF,
                             addr_space="Shared")
    ag2_in = nc.dram_tensor("ag2_in", [AG2], DT_BF)
    ag2_out = nc.dram_tensor("ag2_out", [NC, AG2], DT_BF,
                             addr_space="Shared")

    rg = [list(range(NC))]

    with tile.TileContext(nc) as tc:
        with (
            tc.tile_pool(name="pers", bufs=1) as pers,
            tc.tile_pool(name="gio", bufs=1) as gio,
        ):
            def ptile(shape, dt, tag, bufs=None, pool=None):
                return (pool or pers).tile(shape, dt, tag=tag, name=tag,
                                           bufs=bufs)

            # ---- input loads (dispatch order = DMA priority) -------------
            adjc = ptile([P, NQ * EL], DT_BF, "adjc")
            nc.sync.dma_start(out=adjc[:], in_=adjc_e[:, :])
            featsn = ptile([P, NQ * D], DT_BF, "featsn")
            nc.sync.dma_start(out=featsn[:], in_=featsn_e[:, :])
            wvT_sb = []
            linT_sb = []
            featsTl_sb = []
            for k in range(DKT):
                t = ptile([P, H], DT_BF, f"wvT{k}")
                nc.sync.dma_start(out=t[:], in_=wvT_e[k * P:(k + 1) * P, :])
                wvT_sb.append(t)
                t = ptile([P, H], DT_BF, f"linT{k}")
                nc.sync.dma_start(out=t[:], in_=linT_e[k * P:(k + 1) * P, :])
                linT_sb.append(t)
                t = ptile([P, NL], DT_BF, f"featsTl{k}")
                nc.sync.dma_start(out=t[:],
                                  in_=featsTl_e[k * P:(k + 1) * P, :])
                featsTl_sb.append(t)
            wcol = ptile([H, 1], DT_BF, "wcol")
            nc.sync.dma_start(out=wcol[:], in_=wcol_e[:, :])
            ln2 = ptile([H, 2], DT_F32, "ln2")
            nc.sync.dma_start(out=ln2[:], in_=ln2_e[:, :])
            rows4 = ptile([1, 4 * H], DT_BF, "rows4")
            nc.sync.dma_start(out=rows4[:], in_=rows4_e[:, :])
            c1col = ptile([P, 1], DT_F32, "c1col")
            nc.sync.dma_start(out=c1col[:], in_=c1col_e[:, :])
            n800 = ptile([P, 1], DT_F32, "n800")
            nc.sync.dma_start(out=n800[:], in_=n800_e[:, :])
            ident = ptile([P, P], DT_BF, "ident")
            nc.sync.dma_start(out=ident[:], in_=ident_e[:, :])
            featsTf_sb = []
            for k in range(DKT):
                t = ptile([P, N], DT_BF, f"featsTf{k}")
                nc.sync.dma_start(out=t[:],
                                  in_=featsTf_e[k * P:(k + 1) * P, :])
                featsTf_sb.append(t)

            ones_col = ptile([P, 1], DT_BF, "ones_col")
            nc.vector.memset(ones_col[:], 1.0)
            ones_row = ptile([1, P], DT_BF, "ones_row")
            nc.vector.memset(ones_row[:], 1.0)
            ones8 = ptile([8, P], DT_BF, "ones8")
            nc.vector.memset(ones8[:], 1.0)
            eps_col = ptile([P, 1], DT_F32, "eps_col")
            nc.vector.memset(eps_col[:], LN_EPS)

            # ========= phase 1: e-sharded s path (fully local) ============
            with tc.tile_pool(name="psA1", bufs=1, space="PSUM") as psA:
                # ecT = (adj_shard)^T @ feats = (EL x D), contract all N
                ps_ecT = psA.tile([P, D], DT_F32, tag="ecT", name="ecT",
                                  bufs=1)
                for k in range(NQ):
                    nc.tensor.matmul(
                        ps_ecT[:], lhsT=adjc[:, k * EL:(k + 1) * EL],
                        rhs=featsn[:, k * D:(k + 1) * D],
                        start=(k == 0), stop=(k == NQ - 1))
                ecT_sb = ptile([P, D], DT_BF, "ecT_sb")
                nc.vector.tensor_copy(ecT_sb[:], ps_ecT[:])
                ecd = ptile([P, D], DT_BF, "ecd")   # [128d x 128e] x2 halves
                for dc in range(DKT):
                    pt = psA.tile([P, P], DT_BF, tag="tp", name="tp", bufs=2)
                    nc.tensor.transpose(pt[:],
                                        ecT_sb[:, dc * P:(dc + 1) * P],
                                        ident[:])
                    nc.vector.tensor_copy(ecd[:, dc * P:(dc + 1) * P], pt[:])
                ps_spre = psA.tile([P, H], DT_F32, tag="spre", name="spre",
                                   bufs=1)
                for dc in range(DKT):
                    nc.tensor.matmul(ps_spre[:],
                                     lhsT=ecd[:, dc * P:(dc + 1) * P],
                                     rhs=linT_sb[dc][:],
                                     start=(dc == 0), stop=(dc == DKT - 1))

                # broadcast prep: [w | 2w/800 | ln1w | ln1b]
                ps_bc = psA.tile([P, 4 * H], DT_F32, tag="bc", name="bc",
                                 bufs=1)
                nc.tensor.matmul(ps_bc[:], lhsT=ones_row[:], rhs=rows4[:],
                                 start=True, stop=True)
                bc_sb = ptile([P, 4 * H], DT_BF, "bc_sb")
                nc.vector.tensor_copy(bc_sb[:], ps_bc[:])
                w_bc = bc_sb[:, 0:H]
                w28_bc = bc_sb[:, H:2 * H]
                ln1w_bc = bc_sb[:, 2 * H:3 * H]
                ln1b_bc = bc_sb[:, 3 * H:4 * H]

                # s-LN over free dim on [128e x 64]
                spre_f = ptile([P, H], DT_F32, "spre_f")
                nc.vector.tensor_copy(spre_f[:], ps_spre[:])
                sum1 = ptile([P, 1], DT_F32, "sum1")
                nc.vector.reduce_sum(sum1[:], spre_f[:],
                                     axis=mybir.AxisListType.X)
                nmean1 = ptile([P, 1], DT_F32, "nmean1")
                nc.vector.tensor_scalar(nmean1[:], sum1[:], -1.0 / H, None,
                                        MULT)
                xc = ptile([P, H], DT_F32, "xc")
                nc.vector.tensor_scalar(xc[:], spre_f[:], nmean1[:, :], None,
                                        ADD)
                sq = ptile([P, H], DT_F32, "sq")
                nc.vector.tensor_mul(sq[:], xc[:], xc[:])
                vs1 = ptile([P, 1], DT_F32, "vs1")
                nc.vector.reduce_sum(vs1[:], sq[:], axis=mybir.AxisListType.X)
                sd1 = ptile([P, 1], DT_F32, "sd1")
                nc.scalar.activation(sd1[:], vs1[:], SQRT, scale=1.0 / H,
                                     bias=eps_col[:])
                rstd1 = ptile([P, 1], DT_F32, "rstd1")
                nc.vector.reciprocal_approx_fast(rstd1[:], sd1[:])
                snrm = ptile([P, H], DT_F32, "snrm")
                nc.vector.tensor_scalar(snrm[:], xc[:], rstd1[:, :], None,
                                        MULT)
                s_ln = ptile([P, H], DT_BF, "s_ln")
                nc.vector.tensor_mul(s_ln[:], snrm[:], ln1w_bc)
                nc.vector.tensor_add(s_ln[:], s_ln[:], ln1b_bc)
                # U_r [128e x 66]: [0:64]=2w/800*s_ln, 64=c1-ssw/800, 65=1
                s2 = ptile([P, H], DT_BF, "s2")
                nc.vector.tensor_mul(s2[:], s_ln[:], s_ln[:])
                s2w = ptile([P, H], DT_F32, "s2w")
                nc.vector.tensor_mul(s2w[:], s2[:], w_bc)
                ssw = ptile([P, 1], DT_F32, "ssw")
                nc.vector.reduce_sum(ssw[:], s2w[:],
                                     axis=mybir.AxisListType.X)
                u_r = ptile([P, R], DT_BF, "u_r")
                nc.vector.tensor_mul(u_r[:, 0:H], s_ln[:], w28_bc)
                nc.vector.tensor_scalar(u_r[:, H:H + 1], ssw[:], n800[:, :],
                                        c1col[:, :], MULT, ADD)
                nc.vector.memset(u_r[:, H + 1:R], 1.0)
                ps_us = psA.tile([R, 1], DT_F32, tag="us", name="us", bufs=1)
                nc.tensor.matmul(ps_us[:], lhsT=u_r[:], rhs=ones_col[:],
                                 start=True, stop=True)
                usum_r = ptile([R, 1], DT_BF, "usum_r")
                nc.vector.tensor_copy(usum_r[:], ps_us[:])
                # early u-gather: fires ~t20, absorbs the collective-init
                # barrier + first-op handshake while the d path computes
                nc.sync.dma_start(
                    out=ag1_in[OU:OUS].rearrange("(p f) -> p f", p=P),
                    in_=u_r[:])
                nc.sync.dma_start(
                    out=ag1_in[OUS:AG1].rearrange("(p a) -> p a", p=R),
                    in_=usum_r[:])
                nc.gpsimd.collective_compute(
                    "AllGather", mybir.AluOpType.bypass, replica_groups=rg,
                    ins=[ag1_in[:]], outs=[ag1_out[:, :]])

            # ---- G prefetch (behind critical input loads) ----------------
            g_sb = []
            for m in range(NKT):
                t = gio.tile([P, N], DT_BF, tag=f"gsb{m}", name=f"gsb{m}")
                nc.sync.dma_start(out=t[:],
                                  in_=g_e[m * P:(m + 1) * P, :])
                g_sb.append(t)

            # ========= phase 2: f tiles ===================================
            with tc.tile_pool(name="psA2", bufs=1, space="PSUM") as psA:
                fT_loc = ptile([H, NL], DT_BF, "fT_loc")
                ps_fl = psA.tile([H, NL], DT_F32, tag="ff", name="ff", bufs=2)
                for k in range(DKT):
                    nc.tensor.matmul(ps_fl[:], lhsT=wvT_sb[k][:],
                                     rhs=featsTl_sb[k][:],
                                     start=(k == 0), stop=(k == DKT - 1))
                nc.vector.tensor_copy(fT_loc[:], ps_fl[:])
                fT_full = ptile([H, N], DT_BF, "fT_full")
                for nb in range(NBT):
                    ps_ff = psA.tile([H, NB], DT_F32, tag="ff", name="ff",
                                     bufs=2)
                    for k in range(DKT):
                        nc.tensor.matmul(
                            ps_ff[:], lhsT=wvT_sb[k][:],
                            rhs=featsTf_sb[k][:, nb * NB:(nb + 1) * NB],
                            start=(k == 0), stop=(k == DKT - 1))
                    nc.vector.tensor_copy(fT_full[:, nb * NB:(nb + 1) * NB],
                                          ps_ff[:])
                f_nat = ptile([P, KT * H], DT_BF, "f_nat")
                for k in range(KT):
                    pt = psA.tile([P, H], DT_BF, tag="tp2", name="tp2",
                                  bufs=2)
                    nc.tensor.transpose(pt[:], fT_full[:, k * P:(k + 1) * P],
                                        ident[:H, :H])
                    nc.vector.tensor_copy(f_nat[:, k * H:(k + 1) * H], pt[:])

            # ========= phase 3: exp(scores^T) + dT accumulation ===========
            with tc.tile_pool(name="psB", bufs=1, space="PSUM") as psB:
                ps_dT = psB.tile([H, NL], DT_F32, tag="dT", name="dT", bufs=1)
                for k in range(KT):
                    ps = psB.tile([P, NL], DT_F32, tag="sc", name="sc",
                                  bufs=3)
                    nc.tensor.matmul(ps[:],
                                     lhsT=fT_full[:, k * P:(k + 1) * P],
                                     rhs=fT_loc[:], start=True, stop=True)
                    es = pers.tile([P, NL], DT_BF, tag="es", name="es",
                                   bufs=3)
                    nc.scalar.activation(es[:], ps[:], EXP, scale=0.125)
                    nc.tensor.matmul(ps_dT[:],
                                     lhsT=f_nat[:, k * H:(k + 1) * H],
                                     rhs=es[:],
                                     start=(k == 0), stop=(k == KT - 1))
                dT_pre = ptile([H, NL], DT_BF, "dT_pre")
                nc.vector.tensor_copy(dT_pre[:], ps_dT[:])

            # ========= phase 4: d-LN (partition stats) -> vT -> AllGather =
            vT = ptile([R, NL1], DT_BF, "vT")
            with tc.tile_pool(name="psC", bufs=1, space="PSUM") as psC:
                d2 = ptile([H, NL], DT_BF, "d2")
                nc.vector.tensor_mul(d2[:], dT_pre[:], dT_pre[:])
                ps_srow = psC.tile([1, NL], DT_F32, tag="r1", name="r1",
                                   bufs=2)
                nc.tensor.matmul(ps_srow[:], lhsT=ones_col[:H, :],
                                 rhs=dT_pre[:], start=True, stop=True)
                ps_sqrow = psC.tile([1, NL], DT_F32, tag="r1", name="r1",
                                    bufs=2)
                nc.tensor.matmul(ps_sqrow[:], lhsT=ones_col[:H, :],
                                 rhs=d2[:], start=True, stop=True)
                mean_r = ptile([1, NL], DT_F32, "mean_r")
                nc.vector.tensor_scalar(mean_r[:], ps_srow[:], 1.0 / H, None,
                                        MULT)
                msq_r = ptile([1, NL], DT_F32, "msq_r")
                nc.vector.tensor_mul(msq_r[:], mean_r[:], mean_r[:])
                var_r = ptile([1, NL], DT_F32, "var_r")
                nc.vector.scalar_tensor_tensor(var_r[:], ps_sqrow[:], 1.0 / H,
                                               msq_r[:], MULT, SUB)
                sd_r = ptile([1, NL], DT_F32, "sd_r")
                nc.scalar.activation(sd_r[:], var_r[:], SQRT,
                                     bias=eps_col[:1, :])
                rstd_r = ptile([1, NL], DT_F32, "rstd_r")
                nc.vector.reciprocal_approx_fast(rstd_r[:], sd_r[:])
                ab_row = ptile([1, 2 * NL], DT_BF, "ab_row")
                nc.vector.tensor_copy(ab_row[:, 0:NL], rstd_r[:])
                nc.vector.scalar_tensor_tensor(ab_row[:, NL:2 * NL],
                                               mean_r[:], -1.0, rstd_r[:],
                                               MULT, MULT)
                ps_ab = psC.tile([H, 2 * NL], DT_F32, tag="ab", name="ab",
                                 bufs=1)
                nc.tensor.matmul(ps_ab[:, 0:NL], lhsT=ones_row[:1, :H],
                                 rhs=ab_row[:, 0:NL], start=True, stop=True)
                nc.tensor.matmul(ps_ab[:, NL:2 * NL], lhsT=ones_row[:1, :H],
                                 rhs=ab_row[:, NL:2 * NL], start=True,
                                 stop=True)
                t1 = ptile([H, NL], DT_F32, "t1")
                nc.vector.tensor_mul(t1[:], dT_pre[:], ps_ab[:, 0:NL])
                nc.vector.tensor_add(t1[:], t1[:], ps_ab[:, NL:2 * NL])
                # vT rows 0:64 = d_ln ; row 64 = 1 ; row 65 = -ddw/800 ;
                # col 512 = Vsum_local
                nc.vector.tensor_scalar(vT[0:H, 0:NL], t1[:], ln2[:, 0:1],
                                        ln2[:, 1:2], MULT, ADD)
                nc.vector.memset(vT[H:H + 1, 0:NL], 1.0)
                d2v = ptile([H, NL], DT_BF, "d2v")
                nc.vector.tensor_mul(d2v[:], vT[0:H, 0:NL], vT[0:H, 0:NL])
                ps_dd = psC.tile([1, NL], DT_F32, tag="r1", name="r1", bufs=2)
                nc.tensor.matmul(ps_dd[:], lhsT=wcol[:], rhs=d2v[:],
                                 start=True, stop=True)
                extra1 = ptile([1, NL], DT_BF, "extra1")
                nc.vector.tensor_scalar(extra1[:], ps_dd[:], n800[:1, :],
                                        None, MULT)
                nc.sync.dma_start(out=vT[H + 1:R, 0:NL], in_=extra1[:, :])
                vsum_f = ptile([R, 1], DT_F32, "vsum_f")
                nc.vector.reduce_sum(vsum_f[:], vT[:, 0:NL],
                                     axis=mybir.AxisListType.X)
                nc.vector.tensor_copy(vT[:, NL:NL1], vsum_f[:])

                # local invdv = (Usum . vT)^-1/2 (Usum from ag1; piggyback
                # the row so no per-chunk rsqrt is needed post-ag2)
                usum8 = ptile([8, R], DT_BF, "usum8")
                nc.sync.dma_start(out=usum8[:], in_=ag1_out[:, OUS:AG1])
                ps_usc = psC.tile([R, 1], DT_F32, tag="usc", name="usc",
                                  bufs=1)
                nc.tensor.matmul(ps_usc[:], lhsT=usum8[:],
                                 rhs=ones_col[:8, :], start=True, stop=True)
                usum_col = ptile([R, 1], DT_BF, "usum_col")
                nc.vector.tensor_copy(usum_col[:], ps_usc[:])
                ps_dvl = psC.tile([1, NL], DT_F32, tag="r1", name="r1",
                                  bufs=2)
                nc.tensor.matmul(ps_dvl[:], lhsT=usum_col[:],
                                 rhs=vT[:, 0:NL], start=True, stop=True)
                rdvl = ptile([1, NL], DT_F32, "rdvl")
                nc.vector.reciprocal_approx_fast(rdvl[:], ps_dvl[:])
                invdvl = ptile([1, NL], DT_BF, "invdvl")
                nc.scalar.activation(invdvl[:], rdvl[:], SQRT)

                nc.sync.dma_start(
                    out=ag2_in[OV:OVS].rearrange("(p f) -> p f", p=R),
                    in_=vT[:])
                nc.sync.dma_start(
                    out=ag2_in[OVS:AG2].rearrange("(a f) -> a f", a=1),
                    in_=invdvl[:])
                nc.gpsimd.collective_compute(
                    "AllGather", mybir.AluOpType.bypass, replica_groups=rg,
                    ins=[ag2_in[:]], outs=[ag2_out[:, :]])

            # ========= phase 5: globals from gathers; K, Mt ===============
            # u-side post-processing only needs ag1 (done long ago) and
            # overlaps the ag2 flight.
            with tc.tile_pool(name="psD", bufs=1, space="PSUM") as psD:
                u_full = ptile([P, NC * R], DT_BF, "u_full")
                for r in range(NC):
                    nc.sync.dma_start(
                        out=u_full[:, r * R:(r + 1) * R],
                        in_=ag1_out[r, OU:OUS].rearrange("(p c) -> p c",
                                                         p=P))
                # local Vt = invdv * vT (overlaps ag2 flight)
                ps_dvb = psD.tile([R, NL], DT_F32, tag="dvb", name="dvb",
                                  bufs=2)
                nc.tensor.matmul(ps_dvb[:], lhsT=ones_row[:1, :R],
                                 rhs=invdvl[:], start=True, stop=True)
                vtl = ptile([R, NL], DT_BF, "vtl")
                nc.vector.tensor_mul(vtl[:], vT[:, 0:NL], ps_dvb[:])

                # ---- ag2-dependent part ----------------------------------
                vsum8 = ptile([8, R], DT_BF, "vsum8")
                nc.sync.dma_start(
                    out=vsum8[:].rearrange("r (c a) -> r c a", a=1),
                    in_=ag2_out[:, OV:OVS]
                    .rearrange("r (c z) -> r c z", z=NL1)[:, :, NL:NL1])
                invdv_row = ptile([1, N], DT_BF, "invdv_row")
                nc.sync.dma_start(
                    out=invdv_row[:].rearrange("a (r f) -> a r f", r=NC),
                    in_=ag2_out[:, OVS:AG2].rearrange("r (a f) -> a r f",
                                                      a=1))
                vtf = ptile([R, N], DT_BF, "vtf", pool=gio)
                for r in range(NC):
                    nc.sync.dma_start(
                        out=vtf[:, r * NL:(r + 1) * NL],
                        in_=ag2_out[r, OV:OVS]
                        .rearrange("(c z) -> c z", z=NL1)[:, 0:NL])
                ps_vb = psD.tile([P, R], DT_F32, tag="vb", name="vb", bufs=1)
                nc.tensor.matmul(ps_vb[:], lhsT=ones8[:], rhs=vsum8[:],
                                 start=True, stop=True)
                det = ptile([P, NC * R], DT_F32, "det")
                det3 = det[:].rearrange("p (a b) -> p a b", b=R)
                u3 = u_full[:].rearrange("p (a b) -> p a b", b=R)
                nc.vector.tensor_mul(
                    det3, u3,
                    ps_vb[:].rearrange("p (a b) -> p a b", a=1)
                    .to_broadcast((P, NC, R)))
                de = ptile([P, NC], DT_F32, "de")
                nc.vector.reduce_sum(de[:], det3, axis=mybir.AxisListType.X)
                invde = ptile([P, NC], DT_F32, "invde")
                nc.vector.reciprocal_approx_fast(invde[:], de[:])
                uw = ptile([P, NC * R], DT_BF, "uw")
                uw3 = uw[:].rearrange("p (a b) -> p a b", b=R)
                nc.vector.tensor_mul(
                    uw3, u3,
                    invde[:].rearrange("p (a b) -> p a b", b=1)
                    .to_broadcast((P, NC, R)))
                ps_K = psD.tile([R, R], DT_F32, tag="K", name="K", bufs=1)
                for r in range(NC):
                    nc.tensor.matmul(ps_K[:],
                                     lhsT=uw[:, r * R:(r + 1) * R],
                                     rhs=u_full[:, r * R:(r + 1) * R],
                                     start=(r == 0), stop=(r == NC - 1))
                K001 = ptile([R, R], DT_BF, "K001")
                nc.vector.tensor_scalar(K001[:], ps_K[:], 0.01, None, MULT)
                ps_Mt = psD.tile([R, NL], DT_F32, tag="Mt", name="Mt", bufs=1)
                nc.tensor.matmul(ps_Mt[:], lhsT=K001[:], rhs=vtl[:],
                                 start=True, stop=True)
                Mt = ptile([R, NL], DT_BF, "Mt")
                nc.vector.tensor_copy(Mt[:], ps_Mt[:])

                # global Vt columns: scale vtf by the gathered invdv row
                vts = ptile([R, N], DT_BF, "vts", pool=gio)
                for nb in range(NBT):
                    ps_dvbf = psD.tile([R, NB], DT_F32, tag="dvb",
                                       name="dvb", bufs=2)
                    nc.tensor.matmul(
                        ps_dvbf[:], lhsT=ones_row[:1, :R],
                        rhs=invdv_row[:, nb * NB:(nb + 1) * NB],
                        start=True, stop=True)
                    nc.vector.tensor_mul(vts[:, nb * NB:(nb + 1) * NB],
                                         vtf[:, nb * NB:(nb + 1) * NB],
                                         ps_dvbf[:])

            # ========= phase 6: out = g + M Vt^T ==========================
            with tc.tile_pool(name="psF", bufs=1, space="PSUM") as psF:
                for m in range(NKT):
                    osb = gio.tile([P, N], DT_BF, tag="osb", name="osb",
                                   bufs=2)
                    for nb in range(NBT):
                        ps = psF.tile([P, NB], DT_F32, tag="fin", name="fin",
                                      bufs=4)
                        nc.tensor.matmul(
                            ps[:], lhsT=Mt[:, m * P:(m + 1) * P],
                            rhs=vts[:, nb * NB:(nb + 1) * NB],
                            start=True, stop=True)
                        if nb % 2 == 0:
                            nc.vector.tensor_add(
                                osb[:, nb * NB:(nb + 1) * NB],
                                g_sb[m][:, nb * NB:(nb + 1) * NB], ps[:])
                        else:
                            # offload psum drain to ACT so DVE only does a
                            # cheap bf16+bf16 add
                            gn = gio.tile([P, NB], DT_BF, tag="gn",
                                          name="gn", bufs=3)
                            nc.scalar.copy(gn[:], ps[:])
                            nc.vector.tensor_add(
                                osb[:, nb * NB:(nb + 1) * NB],
                                g_sb[m][:, nb * NB:(nb + 1) * NB], gn[:])
                    nc.sync.dma_start(out=out_e[m * P:(m + 1) * P, :],
                                      in_=osb[:])

            if debug_taps:
                taps = {
                    "d_vT": vT, "d_u_r": u_r, "d_s_ln": s_ln,
                    "d_vtl": vtl, "d_vts": vts, "d_K001": K001,
                    "d_Mt": Mt, "d_de": de, "d_dT_pre": dT_pre,
                    "d_usum8": usum8, "d_vsum8": vsum8,
                    "d_u_full": u_full,
                }
                for nm, t in taps.items():
                    ext = nc.dram_tensor(nm, list(t.shape), t.dtype,
                                         kind="ExternalOutput")
                    nc.sync.dma_start(out=ext[...], in_=t[:])

    nc.compile()
    return nc


_NC_CACHE = None


def _get_nc():
    global _NC_CACHE
    if _NC_CACHE is None:
        _NC_CACHE = build_kernel()
    return _NC_CACHE


def make_in_maps(adj, G, feats, W_v_w, lin_w, w_o_w, w_o_b,
                 ln1_w, ln1_b, ln2_w, ln2_b, kn=None):
    adj = np.asarray(adj, F32)
    G = np.asarray(G, F32)
    feats = np.asarray(feats, F32)
    W_v_w = np.asarray(W_v_w, F32)
    lin_w = np.asarray(lin_w, F32)
    w = np.asarray(w_o_w, F32)[0]
    b = float(np.asarray(w_o_b, F32).reshape(-1)[0])
    ln1_w = np.asarray(ln1_w, F32).reshape(-1)
    ln1_b = np.asarray(ln1_b, F32).reshape(-1)
    ln2_w = np.asarray(ln2_w, F32).reshape(-1)
    ln2_b = np.asarray(ln2_b, F32).reshape(-1)

    g99 = (G * np.float32(0.99)).astype(BF)
    adj_bf = adj.astype(BF)
    feats_bf = feats.astype(BF)
    # row-grouped replicated layouts: partition p holds rows p*NQ..p*NQ+NQ
    featsn = np.ascontiguousarray(
        feats_bf.reshape(P, NQ * D))          # row p*32+q -> [p, q*D:+D]
    featsT_bf = np.ascontiguousarray(feats.T).astype(BF)
    wvT = np.ascontiguousarray(W_v_w.T).astype(BF)
    linT = np.ascontiguousarray(lin_w.T).astype(BF)
    wcol = np.ascontiguousarray(w.reshape(H, 1)).astype(BF)
    ln2 = np.stack([ln2_w, ln2_b], axis=1).astype(F32)
    rows4 = np.concatenate(
        [w, (2.0 / 800.0) * w, ln1_w, ln1_b]).reshape(1, 4 * H).astype(BF)
    c1col = np.full((P, 1), 1.0 - b / 800.0, F32)
    n800 = np.full((P, 1), -1.0 / 800.0, F32)
    ident = np.eye(P, dtype=BF)

    in_maps = []
    for i in range(NC):
        sl = slice(i * NL, (i + 1) * NL)
        esl = slice(i * EL, (i + 1) * EL)
        adjc = np.ascontiguousarray(adj_bf[:, esl]).reshape(P, NQ * EL)
        in_maps.append({
            "adjc": adjc,
            "featsn": featsn,
            "g": np.ascontiguousarray(g99[sl]),
            "featsTf": featsT_bf,
            "featsTl": np.ascontiguousarray(featsT_bf[:, sl]),
            "wvT": wvT,
            "linT": linT,
            "wcol": wcol,
            "ln2": ln2,
            "rows4": rows4,
            "c1col": c1col,
            "n800col": n800,
            "ident": ident,
        })
    return in_maps


def kernel(**inputs) -> np.ndarray:
    nc = _get_nc()
    in_maps = make_in_maps(**inputs)
    res = run_bass_kernel_spmd(nc, in_maps, core_ids=list(range(NC))).results
    return np.concatenate(
        [np.asarray(res[i]["out"]) for i in range(NC)],
        axis=0).astype(np.float32)


if __name__ == "__main__":
    import reference
    inputs = reference.setup_inputs()
    out = kernel(**{k: np.asarray(v) if not np.isscalar(v) else v
                    for k, v in inputs.items()})
    print("out", out.shape, out.dtype)
